# revision 30
# baseline (speedup 1.0000x reference)
"""Causal multi-head attention on 8 Trainium2 NeuronCores.

Problem: B=4, S=2048, D=1024, H=16 heads, d_k=64, causal, fp32 in/out.

Sharding (host side): core c handles batch b=c//2 and head-half hh=c%2
(8 heads = 512 of the 1024 model dims). Each core computes its batch's
attention output for its 8 heads and the partial out-projection through
the matching 512 rows of Wo (+ bo/2, so the pair sums to +bo). The host
gathers by summing the two partials per batch. No collectives needed.

v2: all-bf16 PE path (FWL weight loads), j-major pipelined schedule
with projections / out-projections emitted so the Tile scheduler can
fill attention-phase PE gaps (keeps HAM warm), exact causal trims,
reciprocal via fast custom-DVE op instead of 3.35us InstReciprocal.

On-core layout (PSUM accumulates fp32):
  xT  [1024, 2048]  x[b]^T  bf16                  (host-transposed)
  Q^T, K^T [512, 2048] as 4 tiles [128, 2048]     (head pair per tile)
  V   16 tiles [128 keys, 8 heads x 65] bf16      (65th col = ones -> rowsums)
  scores S^T[k, q] = K^T.T @ Q^T  (contraction d=64; head A at partitions
        0-63, head B at 64-127 -> disjoint PE row groups)
  P^T = exp(0.125 * (S^T + causal mask)) on ACT, straight from PSUM
  AV: out^T[65, q] += V_ext[kb].T @ P^T[kb]  (k-blocks, causally trimmed)
  normalize: row 64 = rowsum -> reciprocal_approx_fast -> gpsimd
        partition_broadcast -> multiply into A^T tiles
  out[s, dm] = A^T.T @ Wo_local + bo/2
"""
import sys

for _p in ("/opt/trn_rl_repo",):
    if _p not in sys.path:
        sys.path.insert(0, _p)

import numpy as np

import concourse.bass as bass
import concourse.tile as tile
from concourse import bacc, bass_utils, library_config, mybir

F32 = mybir.dt.float32
BF16 = mybir.dt.bfloat16
EXPF = mybir.ActivationFunctionType.Exp
ADD = mybir.AluOpType.add
MULT = mybir.AluOpType.mult

D = 1024          # model dim
S = 2048          # sequence length
DL = 512          # local head dims (8 heads x 64)
NH = 8            # local heads
NC_ = 8           # cores
NEG = -1.0e30

_CACHE = {}
TRACE = False
last_results = None


def build_program():
    nc = bacc.Bacc("TRN2", target_bir_lowering=False, debug=False)

    xt_d = nc.dram_tensor("xt", [D, S], BF16, kind="ExternalInput").ap()
    wq_d = nc.dram_tensor("wq", [D, DL], BF16, kind="ExternalInput").ap()
    wk_d = nc.dram_tensor("wk", [D, DL], BF16, kind="ExternalInput").ap()
    wv_d = nc.dram_tensor("wv", [D, DL], BF16, kind="ExternalInput").ap()
    wo_d = nc.dram_tensor("wo", [DL, D], BF16, kind="ExternalInput").ap()
    bq_d = nc.dram_tensor("bq2", [128, 4], F32, kind="ExternalInput").ap()
    bk_d = nc.dram_tensor("bk2", [128, 4], F32, kind="ExternalInput").ap()
    bv_d = nc.dram_tensor("bv", [DL], F32, kind="ExternalInput").ap()
    bo_d = nc.dram_tensor("boh", [D], F32, kind="ExternalInput").ap()
    tri_d = nc.dram_tensor("tri", [128, 128], F32, kind="ExternalInput").ap()
    out_d = nc.dram_tensor("out", [S, D], F32, kind="ExternalOutput").ap()

    xt_r = xt_d.rearrange("(c p) n -> c p n", p=128)    # 8 din-chunks
    wq_r = wq_d.rearrange("(c p) n -> c p n", p=128)
    wk_r = wk_d.rearrange("(c p) n -> c p n", p=128)
    wv_r = wv_d.rearrange("(c p) n -> c p n", p=128)
    wo_r = wo_d.rearrange("(c p) n -> c p n", p=128)    # 4 head-pair chunks

    with tile.TileContext(nc) as tc:
        nc.gpsimd.load_library(library_config.attn)

        consts = tc.alloc_tile_pool(name="consts", bufs=1)

        # ---- constants ----
        tri = consts.tile([128, 128], F32, tag="tri", name="tri")
        nc.sync.dma_start(tri, tri_d)
        bq2 = consts.tile([128, 4], F32, tag="bq2", name="bq2")
        nc.sync.dma_start(bq2, bq_d)
        bk2 = consts.tile([128, 4], F32, tag="bk2", name="bk2")
        nc.sync.dma_start(bk2, bk_d)
        bvb = consts.tile([128, DL], F32, tag="bvb", name="bvb")
        nc.gpsimd.dma_start(
            bvb,
            bass.AP(tensor=bv_d.tensor, offset=bv_d.offset,
                    ap=[[0, 128]] + bv_d.ap))
        bob = consts.tile([128, D], F32, tag="bob", name="bob")
        nc.gpsimd.dma_start(
            bob,
            bass.AP(tensor=bo_d.tensor, offset=bo_d.offset,
                    ap=[[0, 128]] + bo_d.ap))
        ones8 = consts.tile([128, NH], F32, tag="ones8", name="ones8")
        nc.vector.memset(ones8[:], 1.0)

        # ---- persistent data pools (all live together; no phase bars) --
        xtp = tc.alloc_tile_pool(name="xtp", bufs=1)
        xt = [xtp.tile([128, S], BF16, tag=f"xt{i}", name=f"xt{i}")
              for i in range(8)]
        wqp = tc.alloc_tile_pool(name="wqp", bufs=1)
        wqt = [wqp.tile([128, DL], BF16, tag=f"wq{i}", name=f"wq{i}")
               for i in range(8)]
        wkp = tc.alloc_tile_pool(name="wkp", bufs=1)
        wkt = [wkp.tile([128, DL], BF16, tag=f"wk{i}", name=f"wk{i}")
               for i in range(8)]
        wvp = tc.alloc_tile_pool(name="wvp", bufs=1)
        wvt = [wvp.tile([128, DL], BF16, tag=f"wv{i}", name=f"wv{i}")
               for i in range(8)]
        qkp = tc.alloc_tile_pool(name="qkp", bufs=1)
        qt = [qkp.tile([128, S], BF16, tag=f"qt{i}", name=f"qt{i}")
              for i in range(4)]
        kt = [qkp.tile([128, S], BF16, tag=f"kt{i}", name=f"kt{i}")
              for i in range(4)]
        vp = tc.alloc_tile_pool(name="vp", bufs=1)
        v = [vp.tile([128, NH, 65], BF16, tag=f"v{i}", name=f"v{i}")
             for i in range(16)]
        atp = tc.alloc_tile_pool(name="atp", bufs=1)
        at = [atp.tile([128, S], BF16, tag=f"at{i}", name=f"at{i}")
              for i in range(4)]
        wop = tc.alloc_tile_pool(name="wop", bufs=1)
        wo = [wop.tile([128, D], BF16, tag=f"wo{i}", name=f"wo{i}")
              for i in range(4)]
        ptp = tc.alloc_tile_pool(name="ptp", bufs=20)
        rcp = tc.alloc_tile_pool(name="rcp", bufs=2)
        bcp = tc.alloc_tile_pool(name="bcp", bufs=2)
        outp = tc.alloc_tile_pool(name="outp", bufs=3)

        # PSUM: s4p 2x2 banks + avp 2x1 + projp 1 + outpp 1 = 8 banks.
        # proj and outproj get SEPARATE 1-buf pools so their slot rings
        # don't chain (a shared ring serialized proj(j+1) behind
        # outproj(j-1) behind the whole previous attention chunk).
        s4p = tc.alloc_tile_pool(name="s4p", bufs=2, space="PSUM")
        avp = tc.alloc_tile_pool(name="avp", bufs=2, space="PSUM")
        projp = tc.alloc_tile_pool(name="projp", bufs=1, space="PSUM")
        outpp = tc.alloc_tile_pool(name="outpp", bufs=1, space="PSUM")

        # ---- input DMAs: first-task deps first to cut the lead-in ----
        for c in range(8):
            nc.sync.dma_start(wqt[c], wq_r[c])
            nc.sync.dma_start(wkt[c], wk_r[c])
            nc.sync.dma_start(xt[c][:, 0:512], xt_r[c][:, 0:512])
        for c in range(8):
            nc.sync.dma_start(wvt[c], wv_r[c])
        for c in range(8):
            nc.sync.dma_start(xt[c][:, 512:S], xt_r[c][:, 512:S])
        for hc in range(4):
            nc.sync.dma_start(wo[hc], wo_r[hc])

        bvb3 = bvb[:].rearrange("p (h d) -> p h d", h=NH)

        # ================= emitters ==================================
        def emit_qk_proj(j, dc):
            for wts, b2, dst in ((wqt, bq2, qt), (wkt, bk2, kt)):
                ps = projp.tile([128, 512], F32, tag="proj", name="psqk")
                for c in range(8):
                    nc.tensor.matmul(
                        ps[:],
                        wts[c][:, dc * 128:(dc + 1) * 128],
                        xt[c][:, j * 512:(j + 1) * 512],
                        start=(c == 0), stop=(c == 7))
                nc.vector.tensor_scalar_add(
                    dst[dc][:, j * 512:(j + 1) * 512],
                    ps[:], b2[:, dc:dc + 1])

        def emit_v_proj(sb):
            ps = projp.tile([128, 512], F32, tag="proj", name="psv")
            for c in range(8):
                nc.tensor.matmul(
                    ps[:],
                    xt[c][:, sb * 128:(sb + 1) * 128],
                    wvt[c][:],
                    start=(c == 0), stop=(c == 7))
            nc.vector.tensor_tensor(
                v[sb][:, :, 0:64],
                ps[:].rearrange("p (h d) -> p h d", h=NH),
                bvb3, op=ADD)
            nc.vector.tensor_copy(v[sb][:, :, 64], ones8[:])

        def emit_chunk_proj(j):
            for dc in range(4):
                emit_qk_proj(j, dc)
            for sb in range(4 * j, 4 * j + 4):
                emit_v_proj(sb)

        def emit_outproj(j):
            for sb in range(4 * j, 4 * j + 4):
                ot = outp.tile([128, D], F32, tag="ot", name="ot")
                for n in range(2):
                    ps = outpp.tile([128, 512], F32, tag="psd", name="psd")
                    for hc in range(4):
                        nc.tensor.matmul(
                            ps[:],
                            at[hc][:, sb * 128:(sb + 1) * 128],
                            wo[hc][:, n * 512:(n + 1) * 512],
                            start=(hc == 0), stop=(hc == 3))
                    nc.vector.tensor_tensor(
                        ot[:, n * 512:(n + 1) * 512], ps[:],
                        bob[:, n * 512:(n + 1) * 512], op=ADD)
                nc.sync.dma_start(out_d[sb * 128:(sb + 1) * 128, :], ot[:])

        def emit_pass1(dc, j):
            """Scores + exp for all k-blocks of q-chunk j; returns pt tiles."""
            pts = {}              # (g, half) -> pt tile (bf16)
            for g in range(2 * j + 2):   # kb-groups of 2
                s4s = {}
                cs_list = []
                for kk in range(2):
                    kb = 2 * g + kk
                    cs = max(0, 128 * kb - 512 * j)   # exact causal trim
                    cs_list.append((kb, cs))
                # scores: interleave halves so A (rows 0-63) and B
                # (rows 64-127) can overlap in disjoint PE row groups
                for half in range(2):
                    s4s[half] = s4p.tile([128, 1024], F32,
                                         tag="s4", name="s4")
                for kk, (kb, cs) in enumerate(cs_list):
                    for half in range(2):
                        pr = 64 * half
                        nc.tensor.matmul(
                            s4s[half][:, 512 * kk + cs:512 * (kk + 1)],
                            kt[dc][pr:pr + 64, 128 * kb:128 * (kb + 1)],
                            qt[dc][pr:pr + 64, 512 * j + cs:512 * (j + 1)],
                            start=True, stop=True,
                            tile_position=(pr, 0))
                for half in range(2):
                    s4 = s4s[half]
                    for kk, (kb, cs) in enumerate(cs_list):
                        if 128 * kb >= 512 * j:     # diagonal block
                            sl = s4[:, 512 * kk + cs:512 * kk + cs + 128]
                            nc.vector.tensor_tensor(sl, sl, tri[:], op=ADD)
                    cs0 = cs_list[0][1]
                    cs1 = cs_list[1][1]
                    pt = ptp.tile([128, 1024], BF16, tag="pt", name="pt")
                    pts[(g, half)] = pt
                    if cs1 < 352:
                        # merged call; [512:512+cs1) is never-read garbage
                        nc.scalar.activation(
                            pt[:, cs0:1024], s4[:, cs0:1024],
                            EXPF, scale=0.125)
                    else:
                        nc.scalar.activation(
                            pt[:, cs0:512], s4[:, cs0:512],
                            EXPF, scale=0.125)
                        nc.scalar.activation(
                            pt[:, 512 + cs1:1024], s4[:, 512 + cs1:1024],
                            EXPF, scale=0.125)
            return pts

        def emit_pass2(dc, j, pts):
            """One long AV accumulation chain per head + normalize."""
            for half in range(2):
                pr = 64 * half
                av = avp.tile([65, 512], F32, tag="av", name="av")
                for g in range(2 * j + 2):
                    pt = pts[(g, half)]
                    for kk in range(2):
                        kb = 2 * g + kk
                        cs = max(0, 128 * kb - 512 * j)
                        first = (g == 0 and kk == 0)
                        nc.tensor.matmul(
                            av[:, cs:512],
                            v[kb][:, 2 * dc + half, :],
                            pt[:, 512 * kk + cs:512 * (kk + 1)],
                            start=first, stop=True,
                            skip_group_check=not first)
                rsum = rcp.tile([1, 512], F32, tag="rsum", name="rsum")
                nc.vector.tensor_copy(rsum[:], av[64:65, :])
                rec = rcp.tile([1, 512], F32, tag="rec", name="rec")
                nc.vector.reciprocal_approx_fast(rec[:], rsum[:])
                bc = bcp.tile([64, 512], F32, tag="bc", name="bc")
                nc.gpsimd.partition_broadcast(bc[:], rec[:])
                nc.vector.tensor_tensor(
                    at[dc][pr:pr + 64, 512 * j:512 * (j + 1)],
                    av[0:64, :], bc[:], op=MULT)

        # ================= schedule ==================================
        # Chunk-0 projections just-in-time: Q/K for dc emitted right
        # before pass1(0,dc) so the first exp isn't gated on the whole
        # chunk-0 projection block. Later chunks' projections are
        # emitted mid-previous-chunk; outproj AFTER them so PE prefers
        # feeding the next chunk's scores pipeline.
        tasks = [(j, dc) for j in range(4) for dc in range(4)]
        prev = None
        pend_out = None
        for j, dc in tasks:
            if j == 0:
                emit_qk_proj(0, dc)
                if dc == 0:
                    for sb in range(4):
                        emit_v_proj(sb)
            pts = emit_pass1(dc, j)
            if prev is not None:
                emit_pass2(*prev)
                if prev[0] == 3:                 # prev dc == 3
                    pend_out = prev[1]           # chunk prev-j complete
            if dc == 1 and j < 3:
                emit_chunk_proj(j + 1)
            if pend_out is not None and dc >= 1:
                emit_outproj(pend_out)
                pend_out = None
            prev = (dc, j, pts)
        emit_pass2(*prev)
        emit_outproj(3)

        outpp.release()
        projp.release()
        avp.release()
        s4p.release()
        outp.release()
        bcp.release()
        rcp.release()
        ptp.release()
        wop.release()
        atp.release()
        vp.release()
        qkp.release()
        wvp.release()
        wkp.release()
        wqp.release()
        xtp.release()
        consts.release()

    nc.compile()
    return nc


def make_in_maps(x, Wq, bq, Wk, bk, Wv, bv, Wo, bo):
    from ml_dtypes import bfloat16
    x = np.asarray(x, np.float32)
    Wq, bq = np.asarray(Wq, np.float32), np.asarray(bq, np.float32)
    Wk, bk = np.asarray(Wk, np.float32), np.asarray(bk, np.float32)
    Wv, bv = np.asarray(Wv, np.float32), np.asarray(bv, np.float32)
    Wo, bo = np.asarray(Wo, np.float32), np.asarray(bo, np.float32)

    k = np.arange(128)[:, None]
    c = np.arange(128)[None, :]
    tri = np.where(c >= k, 0.0, NEG).astype(np.float32)
    boh = (bo * 0.5).astype(np.float32)

    in_maps = []
    for core in range(NC_):
        b, hh = core // 2, core % 2
        sl = slice(hh * DL, (hh + 1) * DL)
        in_maps.append({
            "xt": np.ascontiguousarray(x[b].T.astype(bfloat16)),
            "wq": np.ascontiguousarray(Wq[:, sl].astype(bfloat16)),
            "wk": np.ascontiguousarray(Wk[:, sl].astype(bfloat16)),
            "wv": np.ascontiguousarray(Wv[:, sl].astype(bfloat16)),
            "wo": np.ascontiguousarray(Wo[sl, :].astype(bfloat16)),
            "bq2": np.ascontiguousarray(bq[sl].reshape(4, 128).T),
            "bk2": np.ascontiguousarray(bk[sl].reshape(4, 128).T),
            "bv": np.ascontiguousarray(bv[sl]),
            "boh": boh,
            "tri": tri,
        })
    return in_maps


def kernel(x, Wq, bq, Wk, bk, Wv, bv, Wo, bo):
    global last_results
    if "nc" not in _CACHE:
        _CACHE["nc"] = build_program()
    nc = _CACHE["nc"]
    in_maps = make_in_maps(x, Wq, bq, Wk, bk, Wv, bv, Wo, bo)
    res = bass_utils.run_bass_kernel_spmd(
        nc, in_maps, core_ids=list(range(NC_)), trace=TRACE)
    last_results = res
    B = 4
    out = np.empty((B, S, D), np.float32)
    for b in range(B):
        out[b] = res.results[2 * b]["out"] + res.results[2 * b + 1]["out"]
    return out
